# revision 8
# baseline (speedup 1.0000x reference)
"""Trainium2 Bass kernel for nn_CustomLinear (rewired linear layer).

The reference computes  out = x @ W.T + bias  plus a per-output-row "rewire"
correction: for rewire e on row r with src column s and clone columns d_k
(denom = K+1), x[:, s] and x[:, d_k] are all replaced by x[:, s]/denom before
the GEMV with weight[r].  Because the correction is linear in x with
coefficients built from the original W, it folds exactly into a modified
weight matrix W':

    dW[r, s]   += (1/denom - 1) * W[r, s] + (1/denom) * sum_k W[r, d_k]
    dW[r, d_k] += -W[r, d_k]
    out = x @ (W + dW).T + bias            (exact, duplicates accumulate)

So the device-side work is one dense GEMM.  Sharding: data-parallel over the
flattened batch axis N across 8 NeuronCores (4096 rows each); W' replicated.
Per core the GEMM runs in fp16 (fp32 PSUM accumulation) at the PE's full
rate; x is pre-packed on the host into a blocked-transposed layout
[32, 128(j), 8(ko), 128(n)] so each x-tile load is one contiguous 256KB DMA
and no on-chip transposes are needed.  The bias add happens on the host
(it is a pure element-wise post-op), and the output is stored fp16 to halve
the store traffic and the end-of-kernel DMA drain.
"""

import sys
import types

import numpy as np

import concourse.bass as bass  # noqa: F401  (bass must import before tile)
import concourse.tile as tile
import concourse.mybir as mybir
from concourse import bacc
from concourse.bass_utils import run_bass_kernel_spmd


def _ensure_ntff_hook():
    """Provide antenv.axon_hooks if the image lacks it.

    run_bass_kernel_spmd(trace=True) (or BASS_TRACE=1) does an unguarded
    `from antenv.axon_hooks import ...`; on images where that module is
    missing the boot skips hook registration silently and a traced run would
    crash.  Registering the shim (plus the ctypes hook when available) makes
    traced runs work and is a no-op for plain runs.
    """
    try:
        import antenv.axon_hooks  # noqa: F401
        return
    except ImportError:
        pass
    mod = types.ModuleType("antenv.axon_hooks")
    _hook = [None]
    mod.set_axon_ntff_profile_hook = lambda h: _hook.__setitem__(0, h)
    mod.get_axon_ntff_profile_hook = lambda: _hook[0]
    sys.modules["antenv.axon_hooks"] = mod
    try:
        import antenv
        antenv.axon_hooks = mod
        from trn_agent_boot.trn_boot import _ntff_profile_via_ctypes
        mod.set_axon_ntff_profile_hook(
            _ntff_profile_via_ctypes('/opt/axon/libaxon_pjrt.so'))
    except Exception:
        pass


_ensure_ntff_hook()

N_CORES = 8
N = 32768
IN_F = 1024
OUT_F = 1024
P = 128
NS = N // N_CORES          # 4096 rows per core
MT = NS // P               # 32 m-tiles per core
KO = IN_F // P             # 8 k-subtiles
OC = 512                   # PSUM free-dim chunk (one fp32 bank)
WARMUP_MM = 30             # junk matmuls bridging the input-DMA window

_nc_cache = None


def _build_nc():
    global _nc_cache
    if _nc_cache is not None:
        return _nc_cache

    nc = bacc.Bacc("TRN2", target_bir_lowering=False, debug=False)
    xb_d = nc.dram_tensor("xb", [MT, P, KO, P], mybir.dt.float16,
                          kind="ExternalInput")
    wt_d = nc.dram_tensor("wt", [KO, P, OUT_F], mybir.dt.float16,
                          kind="ExternalInput")
    out_d = nc.dram_tensor("out", [NS, OUT_F], mybir.dt.float16,
                           kind="ExternalOutput")

    NOC = OUT_F // OC

    with tile.TileContext(nc) as tc:
        with (
            tc.tile_pool(name="wpool", bufs=8) as wpool,
            tc.tile_pool(name="xpool", bufs=5) as xpool,
            tc.tile_pool(name="opool", bufs=3) as opool,
            tc.tile_pool(name="pspool", bufs=4, space="PSUM") as pspool,
            tc.tile_pool(name="warmpool", bufs=1) as warmpool,
            tc.tile_pool(name="warmps", bufs=1, space="PSUM") as warmps,
        ):
            # DMA order: w0, x0, x1, w1..w7, x2, x3.  m0 and m1 run
            # interleaved per-ko during the weight download, so each
            # arriving weight tile feeds 4 matmuls (2 m-tiles x 2 halves)
            # -- the PE consumes weights faster than the DMA delivers them
            # and transitions into the dense stream with no idle gap.
            wts = []
            w0 = wpool.tile([P, OUT_F], mybir.dt.float16, tag="wt",
                            name="wt0")
            nc.sync.dma_start(w0[:], wt_d.ap()[0])
            wts.append(w0)
            xts = {}
            for i in range(2):
                xts[i] = xpool.tile([P, KO, P], mybir.dt.float16, tag="xt",
                                    name=f"xt{i}")
                nc.sync.dma_start(xts[i][:], xb_d.ap()[i])
            for ko in range(1, KO):
                w = wpool.tile([P, OUT_F], mybir.dt.float16, tag="wt",
                               name=f"wt{ko}")
                nc.sync.dma_start(w[:], wt_d.ap()[ko])
                wts.append(w)
            # Pre-issue the next two x-tiles so their DMA issues are not
            # queued behind the m0/m1 store issues on the Sync engine.
            for i in range(2, 4):
                xts[i] = xpool.tile([P, KO, P], mybir.dt.float16, tag="xt",
                                    name=f"xt{i}")
                nc.sync.dma_start(xts[i][:], xb_d.ap()[i])

            # PE warm-up: junk matmuls on a zeroed tile while the input DMAs
            # stream in, so the HAM clock-gate is at 8/8 when the real
            # matmuls start AND stays there through the DMA-limited burst
            # phase (a re-throttle costs ~2x until the next 3.4us window).
            wrm = warmpool.tile([P, P], mybir.dt.float16, tag="wrm")
            nc.gpsimd.memset(wrm[:], 0.0)
            wps = warmps.tile([P, P], mybir.dt.float32, tag="wps")
            for _ in range(WARMUP_MM):
                nc.tensor.matmul(wps[:], wrm[:], wrm[:], start=True, stop=True)

            # Head: m0 + m1 interleaved per-ko.
            hpss = {m: [pspool.tile([P, OC], mybir.dt.float32, tag="ps",
                                    name=f"ps{m}_{oc}")
                        for oc in range(NOC)] for m in range(2)}
            for ko in range(KO):
                for m in range(2):
                    for oc in range(NOC):
                        nc.tensor.matmul(
                            hpss[m][oc][:],
                            xts[m][:, ko, :],
                            wts[ko][:, oc * OC:(oc + 1) * OC],
                            start=(ko == 0),
                            stop=(ko == KO - 1),
                        )
            for m in range(2):
                out_sb = opool.tile([P, OUT_F], mybir.dt.float16, tag="osb")
                for oc in range(NOC):
                    nc.vector.tensor_copy(
                        out_sb[:, oc * OC:(oc + 1) * OC], hpss[m][oc][:])
                nc.sync.dma_start(out_d.ap()[m * P:(m + 1) * P, :], out_sb[:])

            for m in range(2, MT):
                if m < 4:
                    xt = xts[m]
                else:
                    xt = xpool.tile([P, KO, P], mybir.dt.float16, tag="xt")
                    nc.sync.dma_start(xt[:], xb_d.ap()[m])

                out_sb = opool.tile([P, OUT_F], mybir.dt.float16, tag="osb")
                pss = [pspool.tile([P, OC], mybir.dt.float32, tag="ps",
                                   name=f"ps{m}_{oc}")
                       for oc in range(NOC)]
                if m == MT - 1:
                    # Last tile: oc-outer so the first PSUM bank completes
                    # 8 matmuls before the second -- its CAST + store issue
                    # + transfer all overlap the remaining matmuls, leaving
                    # only one quarter-chain after the final matmul.
                    QC = OC // 2
                    for oc in range(NOC):
                        for ko in range(KO):
                            nc.tensor.matmul(
                                pss[oc][:],
                                xt[:, ko, :],
                                wts[ko][:, oc * OC:(oc + 1) * OC],
                                start=(ko == 0),
                                stop=(ko == KO - 1),
                            )
                        for q in range(2):
                            src = pss[oc][:, q * QC:(q + 1) * QC]
                            dst = out_sb[:, oc * OC + q * QC:
                                         oc * OC + (q + 1) * QC]
                            dram = out_d.ap()[m * P:(m + 1) * P,
                                              oc * OC + q * QC:
                                              oc * OC + (q + 1) * QC]
                            if q == 0:
                                nc.vector.tensor_copy(dst, src)
                                nc.sync.dma_start(dram, dst)
                            else:
                                # Scalar both converts and issues its own
                                # store, so the two final quarter-chains run
                                # on disjoint engines in parallel.
                                nc.scalar.copy(dst, src)
                                nc.scalar.dma_start(dram, dst)
                else:
                    # ko-outer: each weight tile feeds both PSUM banks back
                    # to back (one LDWEIGHTS per ko).
                    for ko in range(KO):
                        for oc in range(NOC):
                            nc.tensor.matmul(
                                pss[oc][:],
                                xt[:, ko, :],
                                wts[ko][:, oc * OC:(oc + 1) * OC],
                                start=(ko == 0),
                                stop=(ko == KO - 1),
                            )
                    for oc in range(NOC):
                        nc.vector.tensor_copy(
                            out_sb[:, oc * OC:(oc + 1) * OC], pss[oc][:])
                        if m == MT - 2:
                            # Per-half DMA starts the store as soon as its
                            # PSUM half is evacuated.
                            nc.sync.dma_start(
                                out_d.ap()[m * P:(m + 1) * P,
                                           oc * OC:(oc + 1) * OC],
                                out_sb[:, oc * OC:(oc + 1) * OC])
                    if m < MT - 2:
                        # Steady state: one fully-contiguous 256KB store
                        # (single descriptor chain, best per-queue
                        # throughput).
                        nc.sync.dma_start(out_d.ap()[m * P:(m + 1) * P, :],
                                          out_sb[:])

    nc.compile()
    _nc_cache = nc
    return nc


def _fold_rewires(weight, rewire_rows, rewire_src, rewire_clones):
    """Fold the rewire corrections into the weight matrix (exact, fp32)."""
    r = np.asarray(rewire_rows, dtype=np.int64)
    s = np.asarray(rewire_src, dtype=np.int64)
    d = np.asarray(rewire_clones, dtype=np.int64)
    denom = d.shape[1] + 1
    w_rs = weight[r, s]                      # [R]
    w_rd = weight[r[:, None], d]             # [R, K]
    dW = np.zeros_like(weight)
    np.add.at(dW, (r, s), (1.0 / denom - 1.0) * w_rs + w_rd.sum(axis=1) / denom)
    np.add.at(dW, (r[:, None], d), -w_rd)
    return weight + dW


def _prep_in_maps(x, weight, bias, rewire_rows, rewire_src, rewire_clones):
    """Host-side prep: fold rewires, pack x, build per-core input maps."""
    weight = np.asarray(weight, dtype=np.float32)
    wp = _fold_rewires(weight, rewire_rows, rewire_src, rewire_clones)
    # W'^T in [ko, p(j), o] blocks, fp16.
    wt = np.ascontiguousarray(wp.T).astype(np.float16)
    wt = wt.reshape(KO, P, OUT_F)

    # Pack x: per core [4096, 1024] -> [32, 128(j), 8(ko), 128(n)] fp16.
    xb16 = np.asarray(x, dtype=np.float32).astype(np.float16)
    in_maps = []
    for c in range(N_CORES):
        xs = xb16[c * NS:(c + 1) * NS]
        xbl = np.ascontiguousarray(
            xs.reshape(MT, P, KO, P).transpose(0, 3, 2, 1))
        in_maps.append({"xb": xbl, "wt": wt})
    return in_maps


def kernel(x, weight, bias, rewire_rows, rewire_src, rewire_clones):
    bias = np.asarray(bias, dtype=np.float32)
    in_maps = _prep_in_maps(x, weight, bias, rewire_rows, rewire_src,
                            rewire_clones)
    nc = _build_nc()
    res = run_bass_kernel_spmd(nc, in_maps, list(range(N_CORES)))
    out = np.concatenate([res.results[c]["out"] for c in range(N_CORES)],
                         axis=0)
    return out.astype(np.float32) + bias[None, :]


# revision 9
# speedup vs baseline: 1.0073x; 1.0073x over previous
"""Trainium2 Bass kernel for nn_CustomLinear (rewired linear layer).

The reference computes  out = x @ W.T + bias  plus a per-output-row "rewire"
correction: for rewire e on row r with src column s and clone columns d_k
(denom = K+1), x[:, s] and x[:, d_k] are all replaced by x[:, s]/denom before
the GEMV with weight[r].  Because the correction is linear in x with
coefficients built from the original W, it folds exactly into a modified
weight matrix W':

    dW[r, s]   += (1/denom - 1) * W[r, s] + (1/denom) * sum_k W[r, d_k]
    dW[r, d_k] += -W[r, d_k]
    out = x @ (W + dW).T + bias            (exact, duplicates accumulate)

So the device-side work is one dense GEMM.  Sharding: data-parallel over the
flattened batch axis N across 8 NeuronCores (4096 rows each); W' replicated.
Per core the GEMM runs in fp16 (fp32 PSUM accumulation) at the PE's full
rate; x is pre-packed on the host into a blocked-transposed layout
[32, 128(j), 8(ko), 128(n)] so each x-tile load is one contiguous 256KB DMA
and no on-chip transposes are needed.  The bias add happens on the host
(it is a pure element-wise post-op), and the output is stored fp16 to halve
the store traffic and the end-of-kernel DMA drain.
"""

import sys
import types

import numpy as np

import concourse.bass as bass  # noqa: F401  (bass must import before tile)
import concourse.tile as tile
import concourse.mybir as mybir
from concourse import bacc
from concourse.bass_utils import run_bass_kernel_spmd


def _ensure_ntff_hook():
    """Provide antenv.axon_hooks if the image lacks it.

    run_bass_kernel_spmd(trace=True) (or BASS_TRACE=1) does an unguarded
    `from antenv.axon_hooks import ...`; on images where that module is
    missing the boot skips hook registration silently and a traced run would
    crash.  Registering the shim (plus the ctypes hook when available) makes
    traced runs work and is a no-op for plain runs.
    """
    try:
        import antenv.axon_hooks  # noqa: F401
        return
    except ImportError:
        pass
    mod = types.ModuleType("antenv.axon_hooks")
    _hook = [None]
    mod.set_axon_ntff_profile_hook = lambda h: _hook.__setitem__(0, h)
    mod.get_axon_ntff_profile_hook = lambda: _hook[0]
    sys.modules["antenv.axon_hooks"] = mod
    try:
        import antenv
        antenv.axon_hooks = mod
        from trn_agent_boot.trn_boot import _ntff_profile_via_ctypes
        mod.set_axon_ntff_profile_hook(
            _ntff_profile_via_ctypes('/opt/axon/libaxon_pjrt.so'))
    except Exception:
        pass


_ensure_ntff_hook()

N_CORES = 8
N = 32768
IN_F = 1024
OUT_F = 1024
P = 128
NS = N // N_CORES          # 4096 rows per core
MT = NS // P               # 32 m-tiles per core
KO = IN_F // P             # 8 k-subtiles
OC = 512                   # PSUM free-dim chunk (one fp32 bank)
WARMUP_MM = 30             # junk matmuls bridging the input-DMA window

_nc_cache = None


def _build_nc():
    global _nc_cache
    if _nc_cache is not None:
        return _nc_cache

    nc = bacc.Bacc("TRN2", target_bir_lowering=False, debug=False)
    xb_d = nc.dram_tensor("xb", [MT, P, KO, P], mybir.dt.float16,
                          kind="ExternalInput")
    wt_d = nc.dram_tensor("wt", [KO, P, OUT_F], mybir.dt.float16,
                          kind="ExternalInput")
    out_d = nc.dram_tensor("out", [NS, OUT_F], mybir.dt.float16,
                           kind="ExternalOutput")

    NOC = OUT_F // OC

    with tile.TileContext(nc) as tc:
        with (
            tc.tile_pool(name="wpool", bufs=8) as wpool,
            tc.tile_pool(name="xpool", bufs=5) as xpool,
            tc.tile_pool(name="opool", bufs=3) as opool,
            tc.tile_pool(name="pspool", bufs=4, space="PSUM") as pspool,
            tc.tile_pool(name="warmpool", bufs=1) as warmpool,
            tc.tile_pool(name="warmps", bufs=1, space="PSUM") as warmps,
        ):
            # DMA order: w0, x0, x1, w1..w7, x2, x3.  m0 and m1 run
            # interleaved per-ko during the weight download, so each
            # arriving weight tile feeds 4 matmuls (2 m-tiles x 2 halves)
            # -- the PE consumes weights faster than the DMA delivers them
            # and transitions into the dense stream with no idle gap.
            wts = []
            w0 = wpool.tile([P, OUT_F], mybir.dt.float16, tag="wt",
                            name="wt0")
            nc.sync.dma_start(w0[:], wt_d.ap()[0])
            wts.append(w0)
            xts = {}
            for i in range(2):
                xts[i] = xpool.tile([P, KO, P], mybir.dt.float16, tag="xt",
                                    name=f"xt{i}")
                nc.sync.dma_start(xts[i][:], xb_d.ap()[i])
            for ko in range(1, KO):
                w = wpool.tile([P, OUT_F], mybir.dt.float16, tag="wt",
                               name=f"wt{ko}")
                nc.sync.dma_start(w[:], wt_d.ap()[ko])
                wts.append(w)
            # Pre-issue the next two x-tiles so their DMA issues are not
            # queued behind the m0/m1 store issues on the Sync engine.
            for i in range(2, 4):
                xts[i] = xpool.tile([P, KO, P], mybir.dt.float16, tag="xt",
                                    name=f"xt{i}")
                nc.sync.dma_start(xts[i][:], xb_d.ap()[i])

            # PE warm-up: junk matmuls on a zeroed tile while the input DMAs
            # stream in, so the HAM clock-gate is at 8/8 when the real
            # matmuls start AND stays there through the DMA-limited burst
            # phase (a re-throttle costs ~2x until the next 3.4us window).
            wrm = warmpool.tile([P, P], mybir.dt.float16, tag="wrm")
            nc.gpsimd.memset(wrm[:], 0.0)
            # Pre-warm the scalar engine's activation table (ACT_TABLE_LOAD
            # is ~1.3us and otherwise fires lazily right inside the
            # latency-critical last-tile store chain).
            nc.scalar.copy(wrm[0:1, 0:32], wrm[0:1, 32:64])
            wps = warmps.tile([P, P], mybir.dt.float32, tag="wps")
            for _ in range(WARMUP_MM):
                nc.tensor.matmul(wps[:], wrm[:], wrm[:], start=True, stop=True)

            # Head: m0 + m1 interleaved per-ko.
            hpss = {m: [pspool.tile([P, OC], mybir.dt.float32, tag="ps",
                                    name=f"ps{m}_{oc}")
                        for oc in range(NOC)] for m in range(2)}
            for ko in range(KO):
                for m in range(2):
                    for oc in range(NOC):
                        nc.tensor.matmul(
                            hpss[m][oc][:],
                            xts[m][:, ko, :],
                            wts[ko][:, oc * OC:(oc + 1) * OC],
                            start=(ko == 0),
                            stop=(ko == KO - 1),
                        )
            for m in range(2):
                out_sb = opool.tile([P, OUT_F], mybir.dt.float16, tag="osb")
                for oc in range(NOC):
                    nc.vector.tensor_copy(
                        out_sb[:, oc * OC:(oc + 1) * OC], hpss[m][oc][:])
                nc.sync.dma_start(out_d.ap()[m * P:(m + 1) * P, :], out_sb[:])

            for m in range(2, MT):
                if m < 4:
                    xt = xts[m]
                else:
                    xt = xpool.tile([P, KO, P], mybir.dt.float16, tag="xt")
                    nc.sync.dma_start(xt[:], xb_d.ap()[m])

                out_sb = opool.tile([P, OUT_F], mybir.dt.float16, tag="osb")
                pss = [pspool.tile([P, OC], mybir.dt.float32, tag="ps",
                                   name=f"ps{m}_{oc}")
                       for oc in range(NOC)]
                if m == MT - 1:
                    # Last tile: oc-outer so the first PSUM bank completes
                    # 8 matmuls before the second -- its CAST + store issue
                    # + transfer all overlap the remaining matmuls, leaving
                    # only one quarter-chain after the final matmul.
                    QC = OC // 2
                    for oc in range(NOC):
                        for ko in range(KO):
                            nc.tensor.matmul(
                                pss[oc][:],
                                xt[:, ko, :],
                                wts[ko][:, oc * OC:(oc + 1) * OC],
                                start=(ko == 0),
                                stop=(ko == KO - 1),
                            )
                        for q in range(2):
                            src = pss[oc][:, q * QC:(q + 1) * QC]
                            dst = out_sb[:, oc * OC + q * QC:
                                         oc * OC + (q + 1) * QC]
                            dram = out_d.ap()[m * P:(m + 1) * P,
                                              oc * OC + q * QC:
                                              oc * OC + (q + 1) * QC]
                            if q == 0:
                                nc.vector.tensor_copy(dst, src)
                                nc.sync.dma_start(dram, dst)
                            else:
                                # Scalar both converts and issues its own
                                # store, so the two final quarter-chains run
                                # on disjoint engines in parallel.
                                nc.scalar.copy(dst, src)
                                nc.scalar.dma_start(dram, dst)
                else:
                    # ko-outer: each weight tile feeds both PSUM banks back
                    # to back (one LDWEIGHTS per ko).
                    for ko in range(KO):
                        for oc in range(NOC):
                            nc.tensor.matmul(
                                pss[oc][:],
                                xt[:, ko, :],
                                wts[ko][:, oc * OC:(oc + 1) * OC],
                                start=(ko == 0),
                                stop=(ko == KO - 1),
                            )
                    for oc in range(NOC):
                        nc.vector.tensor_copy(
                            out_sb[:, oc * OC:(oc + 1) * OC], pss[oc][:])
                        if m == MT - 2:
                            # Per-half DMA starts the store as soon as its
                            # PSUM half is evacuated.
                            nc.sync.dma_start(
                                out_d.ap()[m * P:(m + 1) * P,
                                           oc * OC:(oc + 1) * OC],
                                out_sb[:, oc * OC:(oc + 1) * OC])
                    if m < MT - 2:
                        # Steady state: one fully-contiguous 256KB store
                        # (single descriptor chain, best per-queue
                        # throughput).
                        nc.sync.dma_start(out_d.ap()[m * P:(m + 1) * P, :],
                                          out_sb[:])

    nc.compile()
    _nc_cache = nc
    return nc


def _fold_rewires(weight, rewire_rows, rewire_src, rewire_clones):
    """Fold the rewire corrections into the weight matrix (exact, fp32)."""
    r = np.asarray(rewire_rows, dtype=np.int64)
    s = np.asarray(rewire_src, dtype=np.int64)
    d = np.asarray(rewire_clones, dtype=np.int64)
    denom = d.shape[1] + 1
    w_rs = weight[r, s]                      # [R]
    w_rd = weight[r[:, None], d]             # [R, K]
    dW = np.zeros_like(weight)
    np.add.at(dW, (r, s), (1.0 / denom - 1.0) * w_rs + w_rd.sum(axis=1) / denom)
    np.add.at(dW, (r[:, None], d), -w_rd)
    return weight + dW


def _prep_in_maps(x, weight, bias, rewire_rows, rewire_src, rewire_clones):
    """Host-side prep: fold rewires, pack x, build per-core input maps."""
    weight = np.asarray(weight, dtype=np.float32)
    wp = _fold_rewires(weight, rewire_rows, rewire_src, rewire_clones)
    # W'^T in [ko, p(j), o] blocks, fp16.
    wt = np.ascontiguousarray(wp.T).astype(np.float16)
    wt = wt.reshape(KO, P, OUT_F)

    # Pack x: per core [4096, 1024] -> [32, 128(j), 8(ko), 128(n)] fp16.
    xb16 = np.asarray(x, dtype=np.float32).astype(np.float16)
    in_maps = []
    for c in range(N_CORES):
        xs = xb16[c * NS:(c + 1) * NS]
        xbl = np.ascontiguousarray(
            xs.reshape(MT, P, KO, P).transpose(0, 3, 2, 1))
        in_maps.append({"xb": xbl, "wt": wt})
    return in_maps


def kernel(x, weight, bias, rewire_rows, rewire_src, rewire_clones):
    bias = np.asarray(bias, dtype=np.float32)
    in_maps = _prep_in_maps(x, weight, bias, rewire_rows, rewire_src,
                            rewire_clones)
    nc = _build_nc()
    res = run_bass_kernel_spmd(nc, in_maps, list(range(N_CORES)))
    out = np.concatenate([res.results[c]["out"] for c in range(N_CORES)],
                         axis=0)
    return out.astype(np.float32) + bias[None, :]


# revision 10
# speedup vs baseline: 1.0088x; 1.0015x over previous
"""Trainium2 Bass kernel for nn_CustomLinear (rewired linear layer).

The reference computes  out = x @ W.T + bias  plus a per-output-row "rewire"
correction: for rewire e on row r with src column s and clone columns d_k
(denom = K+1), x[:, s] and x[:, d_k] are all replaced by x[:, s]/denom before
the GEMV with weight[r].  Because the correction is linear in x with
coefficients built from the original W, it folds exactly into a modified
weight matrix W':

    dW[r, s]   += (1/denom - 1) * W[r, s] + (1/denom) * sum_k W[r, d_k]
    dW[r, d_k] += -W[r, d_k]
    out = x @ (W + dW).T + bias            (exact, duplicates accumulate)

So the device-side work is one dense GEMM.  Sharding: data-parallel over the
flattened batch axis N across 8 NeuronCores (4096 rows each); W' replicated.
Per core the GEMM runs in fp16 (fp32 PSUM accumulation) at the PE's full
rate; x is pre-packed on the host into a blocked-transposed layout
[32, 128(j), 8(ko), 128(n)] so each x-tile load is one contiguous 256KB DMA
and no on-chip transposes are needed.  The bias add happens on the host
(it is a pure element-wise post-op), and the output is stored fp16 to halve
the store traffic and the end-of-kernel DMA drain.
"""

import sys
import types

import numpy as np

import concourse.bass as bass  # noqa: F401  (bass must import before tile)
import concourse.tile as tile
import concourse.mybir as mybir
from concourse import bacc
from concourse.bass_utils import run_bass_kernel_spmd


def _ensure_ntff_hook():
    """Provide antenv.axon_hooks if the image lacks it.

    run_bass_kernel_spmd(trace=True) (or BASS_TRACE=1) does an unguarded
    `from antenv.axon_hooks import ...`; on images where that module is
    missing the boot skips hook registration silently and a traced run would
    crash.  Registering the shim (plus the ctypes hook when available) makes
    traced runs work and is a no-op for plain runs.
    """
    try:
        import antenv.axon_hooks  # noqa: F401
        return
    except ImportError:
        pass
    mod = types.ModuleType("antenv.axon_hooks")
    _hook = [None]
    mod.set_axon_ntff_profile_hook = lambda h: _hook.__setitem__(0, h)
    mod.get_axon_ntff_profile_hook = lambda: _hook[0]
    sys.modules["antenv.axon_hooks"] = mod
    try:
        import antenv
        antenv.axon_hooks = mod
        from trn_agent_boot.trn_boot import _ntff_profile_via_ctypes
        mod.set_axon_ntff_profile_hook(
            _ntff_profile_via_ctypes('/opt/axon/libaxon_pjrt.so'))
    except Exception:
        pass


_ensure_ntff_hook()

N_CORES = 8
N = 32768
IN_F = 1024
OUT_F = 1024
P = 128
NS = N // N_CORES          # 4096 rows per core
MT = NS // P               # 32 m-tiles per core
KO = IN_F // P             # 8 k-subtiles
OC = 512                   # PSUM free-dim chunk (one fp32 bank)
WARMUP_MM = 22             # junk matmuls bridging the input-DMA window

_nc_cache = None


def _build_nc():
    global _nc_cache
    if _nc_cache is not None:
        return _nc_cache

    nc = bacc.Bacc("TRN2", target_bir_lowering=False, debug=False)
    xb_d = nc.dram_tensor("xb", [MT, P, KO, P], mybir.dt.float16,
                          kind="ExternalInput")
    wt_d = nc.dram_tensor("wt", [KO, P, OUT_F], mybir.dt.float16,
                          kind="ExternalInput")
    out_d = nc.dram_tensor("out", [NS, OUT_F], mybir.dt.float16,
                           kind="ExternalOutput")

    NOC = OUT_F // OC

    with tile.TileContext(nc) as tc:
        with (
            tc.tile_pool(name="wpool", bufs=8) as wpool,
            tc.tile_pool(name="xpool", bufs=7) as xpool,
            tc.tile_pool(name="opool", bufs=3) as opool,
            tc.tile_pool(name="pspool", bufs=4, space="PSUM") as pspool,
            tc.tile_pool(name="warmpool", bufs=1) as warmpool,
            tc.tile_pool(name="warmps", bufs=1, space="PSUM") as warmps,
        ):
            # DMA order: w0, x0, x1, w1..w7, x2, x3.  m0 and m1 run
            # interleaved per-ko during the weight download, so each
            # arriving weight tile feeds 4 matmuls (2 m-tiles x 2 halves)
            # -- the PE consumes weights faster than the DMA delivers them
            # and transitions into the dense stream with no idle gap.
            wts = []
            w0 = wpool.tile([P, OUT_F], mybir.dt.float16, tag="wt",
                            name="wt0")
            nc.sync.dma_start(w0[:], wt_d.ap()[0])
            wts.append(w0)
            xts = {}
            for i in range(2):
                xts[i] = xpool.tile([P, KO, P], mybir.dt.float16, tag="xt",
                                    name=f"xt{i}")
                nc.sync.dma_start(xts[i][:], xb_d.ap()[i])
            for ko in range(1, KO):
                w = wpool.tile([P, OUT_F], mybir.dt.float16, tag="wt",
                               name=f"wt{ko}")
                nc.sync.dma_start(w[:], wt_d.ap()[ko])
                wts.append(w)
            # Pre-issue the next four x-tiles so their DMA issues (and
            # packets) are not queued behind the m0/m1 store issues.
            for i in range(2, 6):
                xts[i] = xpool.tile([P, KO, P], mybir.dt.float16, tag="xt",
                                    name=f"xt{i}")
                nc.sync.dma_start(xts[i][:], xb_d.ap()[i])

            # PE warm-up: junk matmuls on a zeroed tile while the input DMAs
            # stream in, so the HAM clock-gate is at 8/8 when the real
            # matmuls start AND stays there through the DMA-limited burst
            # phase (a re-throttle costs ~2x until the next 3.4us window).
            wrm = warmpool.tile([P, P], mybir.dt.float16, tag="wrm")
            nc.gpsimd.memset(wrm[:], 0.0)
            # Pre-warm the scalar engine's activation table (ACT_TABLE_LOAD
            # is ~1.3us and otherwise fires lazily right inside the
            # latency-critical last-tile store chain).
            nc.scalar.copy(wrm[0:1, 0:32], wrm[0:1, 32:64])
            wps = warmps.tile([P, P], mybir.dt.float32, tag="wps")
            for _ in range(WARMUP_MM):
                nc.tensor.matmul(wps[:], wrm[:], wrm[:], start=True, stop=True)

            # Head: m0 + m1 interleaved per-ko.
            hpss = {m: [pspool.tile([P, OC], mybir.dt.float32, tag="ps",
                                    name=f"ps{m}_{oc}")
                        for oc in range(NOC)] for m in range(2)}
            for ko in range(KO):
                for m in range(2):
                    for oc in range(NOC):
                        nc.tensor.matmul(
                            hpss[m][oc][:],
                            xts[m][:, ko, :],
                            wts[ko][:, oc * OC:(oc + 1) * OC],
                            start=(ko == 0),
                            stop=(ko == KO - 1),
                        )
            for m in range(2):
                out_sb = opool.tile([P, OUT_F], mybir.dt.float16, tag="osb")
                for oc in range(NOC):
                    nc.vector.tensor_copy(
                        out_sb[:, oc * OC:(oc + 1) * OC], hpss[m][oc][:])
                nc.sync.dma_start(out_d.ap()[m * P:(m + 1) * P, :], out_sb[:])

            for m in range(2, MT):
                if m < 6:
                    xt = xts[m]
                else:
                    xt = xpool.tile([P, KO, P], mybir.dt.float16, tag="xt")
                    nc.sync.dma_start(xt[:], xb_d.ap()[m])

                out_sb = opool.tile([P, OUT_F], mybir.dt.float16, tag="osb")
                pss = [pspool.tile([P, OC], mybir.dt.float32, tag="ps",
                                   name=f"ps{m}_{oc}")
                       for oc in range(NOC)]
                if m == MT - 1:
                    # Last tile: oc-outer so the first PSUM bank completes
                    # 8 matmuls before the second -- its CAST + store issue
                    # + transfer all overlap the remaining matmuls, leaving
                    # only one quarter-chain after the final matmul.
                    QC = OC // 2
                    for oc in range(NOC):
                        for ko in range(KO):
                            nc.tensor.matmul(
                                pss[oc][:],
                                xt[:, ko, :],
                                wts[ko][:, oc * OC:(oc + 1) * OC],
                                start=(ko == 0),
                                stop=(ko == KO - 1),
                            )
                        for q in range(2):
                            src = pss[oc][:, q * QC:(q + 1) * QC]
                            dst = out_sb[:, oc * OC + q * QC:
                                         oc * OC + (q + 1) * QC]
                            dram = out_d.ap()[m * P:(m + 1) * P,
                                              oc * OC + q * QC:
                                              oc * OC + (q + 1) * QC]
                            if q == 0:
                                nc.vector.tensor_copy(dst, src)
                                nc.sync.dma_start(dram, dst)
                            else:
                                # Scalar both converts and issues its own
                                # store, so the two final quarter-chains run
                                # on disjoint engines in parallel.
                                nc.scalar.copy(dst, src)
                                nc.scalar.dma_start(dram, dst)
                else:
                    # ko-outer: each weight tile feeds both PSUM banks back
                    # to back (one LDWEIGHTS per ko).
                    for ko in range(KO):
                        for oc in range(NOC):
                            nc.tensor.matmul(
                                pss[oc][:],
                                xt[:, ko, :],
                                wts[ko][:, oc * OC:(oc + 1) * OC],
                                start=(ko == 0),
                                stop=(ko == KO - 1),
                            )
                    for oc in range(NOC):
                        nc.vector.tensor_copy(
                            out_sb[:, oc * OC:(oc + 1) * OC], pss[oc][:])
                        if m == MT - 2:
                            # Per-half DMA starts the store as soon as its
                            # PSUM half is evacuated.
                            nc.sync.dma_start(
                                out_d.ap()[m * P:(m + 1) * P,
                                           oc * OC:(oc + 1) * OC],
                                out_sb[:, oc * OC:(oc + 1) * OC])
                    if m < MT - 2:
                        # Steady state: one fully-contiguous 256KB store
                        # (single descriptor chain, best per-queue
                        # throughput).
                        nc.sync.dma_start(out_d.ap()[m * P:(m + 1) * P, :],
                                          out_sb[:])

    nc.compile()
    _nc_cache = nc
    return nc


def _fold_rewires(weight, rewire_rows, rewire_src, rewire_clones):
    """Fold the rewire corrections into the weight matrix (exact, fp32)."""
    r = np.asarray(rewire_rows, dtype=np.int64)
    s = np.asarray(rewire_src, dtype=np.int64)
    d = np.asarray(rewire_clones, dtype=np.int64)
    denom = d.shape[1] + 1
    w_rs = weight[r, s]                      # [R]
    w_rd = weight[r[:, None], d]             # [R, K]
    dW = np.zeros_like(weight)
    np.add.at(dW, (r, s), (1.0 / denom - 1.0) * w_rs + w_rd.sum(axis=1) / denom)
    np.add.at(dW, (r[:, None], d), -w_rd)
    return weight + dW


def _prep_in_maps(x, weight, bias, rewire_rows, rewire_src, rewire_clones):
    """Host-side prep: fold rewires, pack x, build per-core input maps."""
    weight = np.asarray(weight, dtype=np.float32)
    wp = _fold_rewires(weight, rewire_rows, rewire_src, rewire_clones)
    # W'^T in [ko, p(j), o] blocks, fp16.
    wt = np.ascontiguousarray(wp.T).astype(np.float16)
    wt = wt.reshape(KO, P, OUT_F)

    # Pack x: per core [4096, 1024] -> [32, 128(j), 8(ko), 128(n)] fp16.
    xb16 = np.asarray(x, dtype=np.float32).astype(np.float16)
    in_maps = []
    for c in range(N_CORES):
        xs = xb16[c * NS:(c + 1) * NS]
        xbl = np.ascontiguousarray(
            xs.reshape(MT, P, KO, P).transpose(0, 3, 2, 1))
        in_maps.append({"xb": xbl, "wt": wt})
    return in_maps


def kernel(x, weight, bias, rewire_rows, rewire_src, rewire_clones):
    bias = np.asarray(bias, dtype=np.float32)
    in_maps = _prep_in_maps(x, weight, bias, rewire_rows, rewire_src,
                            rewire_clones)
    nc = _build_nc()
    res = run_bass_kernel_spmd(nc, in_maps, list(range(N_CORES)))
    out = np.concatenate([res.results[c]["out"] for c in range(N_CORES)],
                         axis=0)
    return out.astype(np.float32) + bias[None, :]
